# revision 1
# baseline (speedup 1.0000x reference)
"""DiffFDN Trainium2 kernel, v3: DRAM history + indirect gathers.

Per core (4 items): the 48000-step FDN scan becomes 94 blocks of
[64x68]^T @ [64x500] (float32r). History lives in DRAM as one
[68, TPAD] tensor (rows 0-63 per-(line,item) nxt series, rows 64-67 the
y output series). Per block: one PSUM->SBUF copy, one staged HWDGE
write to DRAM, one SWDGE *indirect* gather (per-row flat offsets) that
assembles the 16 time-shifted line reads in a single instruction.

The gather's in_ AP is the column-sliced prefix H[:, 0:PAD+n0-500] so
the Tile dependency tracker sees the true dependency (the write from
block b-2), keeping a 2-deep software pipeline; offsets are view-flat
element indices r*X_b + (PAD + n0 - d_i).
"""

import numpy as np

SR = 48000
IR_LEN = 48000
DELAYS = [1009, 1123, 1231, 1321, 1433, 1543, 1657, 1777, 1879, 1987,
          2081, 2179, 2287, 2383, 2503, 2617]
N = 16
FEAT = 256
BATCH = 32
NCORES = 8
IPC = BATCH // NCORES
L = 500
PAD = 2620                     # zero padding before t=0 (>= max delay)
TPAD = PAD + IR_LEN + 500
NBLK = IR_LEN // L             # 96; blocks 0,1 skipped (all-zero)
M_OUT = IPC * N + IPC          # 68

_BUILT = None
K_DEP = 2          # gather only covers blocks <= b-K_DEP (pipeline depth)


def _patch_list():
    """Pieces of each line's read window sourced from blocks > b-K_DEP.

    These are patched into S from the SBUF stage tiles (the DRAM gather
    raced/skipped those regions). Returns (line, rel_blk, src_col,
    dst_col, length) tuples; rel_blk is source block minus b.
    """
    out = []
    for i in range(N):
        d = DELAYS[i]
        lo, hi = -d, -d + L
        a = lo
        while a < hi:
            e = min(hi, (a // L + 1) * L)
            rel = a // L
            if rel >= -(K_DEP - 1):
                out.append((i, rel, a - rel * L, a - lo, e - a))
            a = e
    return out


def _expm64(M):
    M = M.astype(np.float64)
    nrm = np.linalg.norm(M, ord=np.inf)
    k = max(0, int(np.ceil(np.log2(max(nrm, 1e-30)))) + 2)
    Ms = M / (2.0 ** k)
    E = np.eye(M.shape[0]) + Ms
    term = Ms.copy()
    for i in range(2, 18):
        term = term @ Ms / i
        E = E + term
    for _ in range(k):
        E = E @ E
    return E


def _prologue(x, WA, bA, WB, bB, WC, bC):
    x = np.asarray(x, np.float32)
    feat = x.mean(axis=1)
    A = np.tanh(feat @ np.asarray(WA).T + bA).reshape(-1, N, N)
    Bv = np.tanh(feat @ np.asarray(WB).T + bB)
    Cv = np.tanh(feat @ np.asarray(WC).T + bC)
    S = np.triu(A, 1)
    S = S - np.swapaxes(S, -1, -2)
    g = 10.0 ** (-3.0 / SR)
    G = g ** np.asarray(DELAYS, np.float64)
    A_g = np.stack([_expm64(S[b]) for b in range(S.shape[0])])
    A_g = (A_g * G[None, None, :]).astype(np.float32)
    return A_g, Bv.astype(np.float32), Cv.astype(np.float32)


def _core_inputs(A_g4, Bv4, Cv4):
    lhsT = np.zeros((IPC * N, M_OUT), np.float32)
    bv = np.zeros((IPC * N, 1), np.float32)
    for j in range(IPC):
        for i in range(N):
            r = 4 * i + j
            for ip in range(N):
                lhsT[r, 4 * ip + j] = A_g4[j, ip, i]
            lhsT[r, IPC * N + j] = Cv4[j, i]
            bv[r, 0] = Bv4[j, i]
    return lhsT, bv


OFFS_PHYSICAL = True


def _offsets():
    """offs[r, b-2] = flat gather offset for row r, sub-block b.

    OFFS_PHYSICAL: offsets are element offsets into the physical tensor
    (row stride TPAD) -- what the HW descriptor generator uses. CoreSim
    instead flattens the sliced view (row stride X_b).
    """
    offs = np.zeros((IPC * N, NBLK - 2), np.uint32)
    for b in range(2, NBLK):
        n0 = L * b
        Xb = TPAD if OFFS_PHYSICAL else (PAD + n0 - L)
        for i in range(N):
            for j in range(IPC):
                r = 4 * i + j
                offs[r, b - 2] = r * Xb + (PAD + n0 - DELAYS[i])
    return offs


def _build():
    global _BUILT
    if _BUILT is not None:
        return _BUILT
    import concourse.bacc as bacc
    import concourse.bass as bass
    import concourse.mybir as mybir
    import concourse.tile as tile

    fp32 = mybir.dt.float32
    f32r = mybir.dt.float32r
    u32 = mybir.dt.uint32
    nc = bacc.Bacc("TRN2", target_bir_lowering=False, debug=False)
    lhsT_d = nc.dram_tensor("lhsT", [IPC * N, M_OUT], f32r, kind="ExternalInput")
    bv_d = nc.dram_tensor("bv", [IPC * N, 1], f32r, kind="ExternalInput")
    offs_d = nc.dram_tensor("offs", [IPC * N, NBLK - 2], u32, kind="ExternalInput")
    patches = _patch_list()
    npat = len(patches)
    pmask_d = None
    if npat:
        pmask_d = nc.dram_tensor(
            "pmask", [IPC * N, npat], mybir.dt.uint8, kind="ExternalInput")
    y_d = nc.dram_tensor("y", [IPC, IR_LEN], f32r, kind="ExternalOutput")
    h_d = nc.dram_tensor("hist", [M_OUT, TPAD], f32r)

    with tile.TileContext(nc) as tc:
        with tc.tile_pool(name="const", bufs=1) as cpool, \
             tc.tile_pool(name="init", bufs=1) as ipool, \
             tc.tile_pool(name="sg", bufs=8) as spool, \
             tc.tile_pool(name="st", bufs=10) as tpool, \
             tc.tile_pool(name="ps", bufs=8, space="PSUM") as ppool, \
             tc.tile_pool(name="yb", bufs=2) as ypool:
            lhsT = cpool.tile([IPC * N, M_OUT], f32r)
            nc.sync.dma_start(lhsT[:, :], lhsT_d[:, :])
            offs = cpool.tile([IPC * N, NBLK - 2], u32)
            nc.sync.dma_start(offs[:, :], offs_d[:, :])
            pmask = None
            if npat:
                pmask = cpool.tile([IPC * N, npat], mybir.dt.uint8)
                nc.sync.dma_start(pmask[:, :], pmask_d[:, :])

            # zero-init history cols [0, PAD+1000) incl. y rows; Bv impulse
            # lands at col PAD (time 0) via the same staged image.
            z = ipool.tile([M_OUT, PAD + 2 * L], fp32)
            half = (PAD + 2 * L) // 2
            nc.vector.memset(z[:, 0:half], 0.0)
            nc.gpsimd.memset(z[:, half:], 0.0)
            nc.sync.dma_start(z[0:IPC * N, PAD:PAD + 1].bitcast(f32r), bv_d[:, :])
            nc.scalar.dma_start(
                h_d[:, 0:PAD + 2 * L].bitcast(fp32), z[:, :])

            stages = {}  # b -> (tile, col0) holding that block's nxt in SBUF
            stages[-1] = (z, PAD - L)   # negative time: zeros
            stages[0] = (z, PAD)
            stages[1] = (z, PAD + L)
            for b in range(2, NBLK):
                n0 = L * b
                Xb = PAD + n0 - (K_DEP - 1) * L
                S = spool.tile([IPC * N, L], f32r)
                nc.gpsimd.indirect_dma_start(
                    out=S[:, :], out_offset=None,
                    in_=h_d[0:IPC * N, 0:Xb],
                    in_offset=bass.IndirectOffsetOnAxis(
                        ap=offs[:, b - 2:b - 1], axis=1),
                )
                # patch recent-sourced pieces of S from the SBUF stages
                # (the DRAM gather raced/skipped those regions). Engines
                # require 32-aligned partition bases, so each patch runs
                # base-0 over all rows with a per-line row mask.
                for k, (i, rel, sc, dc, ln) in enumerate(patches):
                    ptile, pcol = stages[b + rel]
                    src = ptile[0:IPC * N, pcol + sc:pcol + sc + ln]
                    if src.dtype != f32r:
                        src = src.bitcast(f32r)
                    nc.vector.copy_predicated(
                        S[:, dc:dc + ln],
                        pmask[:, k:k + 1].to_broadcast([IPC * N, ln]),
                        src,
                    )
                ps = ppool.tile([M_OUT, L], fp32)
                nc.tensor.matmul(ps[:, :], lhsT[:, :], S[:, :],
                                 start=True, stop=True)
                stage = tpool.tile([M_OUT, L], f32r)
                stages[b] = (stage, 0)
                if b % 2 == 0:
                    nc.vector.tensor_copy(stage[:, :], ps[:, :])
                else:
                    nc.scalar.copy(stage[:, :], ps[:, :])
                weng = nc.sync if b % 2 == 0 else nc.scalar
                weng.dma_start(h_d[:, PAD + n0:PAD + n0 + L], stage[:, :])

                # y extraction (hist rows 64..67 -> y, bounced via SBUF),
                # interleaved: chunk k is final once block 24*(k+1) has
                # been written, so it overlaps the remaining compute
                # instead of trailing the last block.
                CH = 12000
                if b >= 25 and (b - 25) % 24 == 0 and (k := (b - 25) // 24) < 3:
                    yb = ypool.tile([IPC, CH], f32r)
                    nc.scalar.dma_start(
                        yb[:, :],
                        h_d[IPC * N:M_OUT, PAD + k * CH:PAD + (k + 1) * CH])
                    nc.scalar.dma_start(y_d[:, k * CH:(k + 1) * CH], yb[:, :])
                # last two blocks: ship y straight from the SBUF stage so
                # the kernel tail doesn't wait on their DRAM writes
                if b >= NBLK - 2:
                    nc.sync.dma_start(
                        y_d[:, n0:n0 + L], stage[IPC * N:M_OUT, :])

            # remaining y span [3*CH, last two blocks) after the loop;
            # reads only blocks <= NBLK-3 so it overlaps the loop tail
            yb = ypool.tile([IPC, CH], f32r)
            span = IR_LEN - L * 2 - 3 * CH
            nc.scalar.dma_start(
                yb[:, 0:span],
                h_d[IPC * N:M_OUT, PAD + 3 * CH:PAD + 3 * CH + span])
            nc.scalar.dma_start(y_d[:, 3 * CH:3 * CH + span], yb[:, 0:span])
    nc.compile()
    _BUILT = nc
    return nc


def _pmask():
    pl = _patch_list()
    pm = np.zeros((IPC * N, len(pl)), np.uint8)
    for k, (i, _, _, _, _) in enumerate(pl):
        pm[4 * i:4 * i + 4, k] = 1
    return pm


def kernel(x, WA, bA, WB, bB, WC, bC):
    from concourse import bass_utils

    A_g, Bv, Cv = _prologue(x, WA, bA, WB, bB, WC, bC)
    offs = _offsets()
    pm = _pmask()
    in_maps = []
    for k in range(NCORES):
        sl = slice(k * IPC, (k + 1) * IPC)
        lhsT, bv = _core_inputs(A_g[sl], Bv[sl], Cv[sl])
        m = {"lhsT": lhsT, "bv": bv, "offs": offs}
        if pm.shape[1]:
            m["pmask"] = pm
        in_maps.append(m)

    nc = _build()
    res = bass_utils.run_bass_kernel_spmd(nc, in_maps, core_ids=list(range(NCORES)))
    y = np.concatenate([res.results[k]["y"] for k in range(NCORES)], axis=0)
    return y[:, None, :].astype(np.float32)



# revision 12
# speedup vs baseline: 1.2652x; 1.2652x over previous
"""DiffFDN Trainium2 kernel, v6: deep-lag gather + ring-fed PE/engine patches.

Spread 128-row layout so near-delay lines sit at 32-aligned partition bases:
engine patches become plain row-sliced copies (legal bases 0/32/64/96), and
f32r outputs work (InstTensorCopy rounds; InstCopyPredicated cannot).

Rows: line0@0-3, line1@4-7 (PE lines), line2@32-35, line3@64-67, line4@96-99,
far lines 5-10 @ 8-31, 11-15 @ 36-55, y @ 100-103; rest unused (lhsT zeros).

Per block b (L=500, 94 active blocks):
- SWDGE indirect gather of S[128,500] from DRAM hist; its tracked dep is
  write(b-3) (K_DEP=3), so the DRAM roundtrip spans 3 blocks. Stale block-
  (b-2) columns are overwritten by:
- plain-copy patches for lines 2-4 (suffix widths 269/179/67) sourced from
  the SBUF stage ring slot of block b-2, and
- lines 0,1 dropped from the main lhsT entirely: two extra accumulating
  matmuls read their full 500-wide windows straight from the contiguous
  stage ring (9 slots x 500 + 500-col mirror of slot 0 for wraparound).
- main matmul (stop=True) + PSUM->ring copy + HWDGE write to hist + y DMAs.
"""

import numpy as np

SR = 48000
IR_LEN = 48000
DELAYS = [1009, 1123, 1231, 1321, 1433, 1543, 1657, 1777, 1879, 1987,
          2081, 2179, 2287, 2383, 2503, 2617]
N = 16
FEAT = 256
BATCH = 32
NCORES = 8
IPC = BATCH // NCORES
L = 500
PAD = 2620
TPAD = PAD + IR_LEN + 500
NBLK = IR_LEN // L             # 96; blocks 0,1 skipped (all-zero)
NR = 128                       # spread row count

_BUILT = None
K_DEP = 3
PE_LINES = (0, 1)
ROWB = {0: 0, 1: 4, 2: 32, 3: 64, 4: 96, 5: 8, 6: 12, 7: 16, 8: 20, 9: 24,
        10: 28, 11: 36, 12: 40, 13: 44, 14: 48, 15: 52}
YR = 100
# engine per patch line: 2 -> ACT, 3 -> DVE, 4 -> DVE
PATCH_ENG = {2: "scalar", 3: "vector", 4: "vector"}


def _line_pieces(i):
    d = DELAYS[i]
    lo, hi = -d, -d + L
    out = []
    a = lo
    while a < hi:
        e = min(hi, (a // L + 1) * L)
        rel = a // L
        out.append((rel, a - rel * L, a - lo, e - a))
        a = e
    return out


def _patch_list():
    """(line, rel, src_col, dst_col, len) for stale pieces of non-PE lines."""
    out = []
    for i in range(N):
        if i in PE_LINES:
            continue
        for (rel, sc, dc, ln) in _line_pieces(i):
            if rel >= -(K_DEP - 1):
                out.append((i, rel, sc, dc, ln))
    return out


def _expm64(M):
    M = M.astype(np.float64)
    nrm = np.linalg.norm(M, ord=np.inf)
    k = max(0, int(np.ceil(np.log2(max(nrm, 1e-30)))) + 2)
    Ms = M / (2.0 ** k)
    E = np.eye(M.shape[0]) + Ms
    term = Ms.copy()
    for i in range(2, 18):
        term = term @ Ms / i
        E = E + term
    for _ in range(k):
        E = E @ E
    return E


def _prologue(x, WA, bA, WB, bB, WC, bC):
    x = np.asarray(x, np.float32)
    feat = x.mean(axis=1)
    A = np.tanh(feat @ np.asarray(WA).T + bA).reshape(-1, N, N)
    Bv = np.tanh(feat @ np.asarray(WB).T + bB)
    Cv = np.tanh(feat @ np.asarray(WC).T + bC)
    S = np.triu(A, 1)
    S = S - np.swapaxes(S, -1, -2)
    g = 10.0 ** (-3.0 / SR)
    G = g ** np.asarray(DELAYS, np.float64)
    A_g = np.stack([_expm64(S[b]) for b in range(S.shape[0])])
    A_g = (A_g * G[None, None, :]).astype(np.float32)
    return A_g, Bv.astype(np.float32), Cv.astype(np.float32)


def _core_inputs(A_g4, Bv4, Cv4):
    lhsT = np.zeros((NR, NR), np.float32)
    bv = np.zeros((NR, 1), np.float32)
    for j in range(IPC):
        for i in range(N):
            r = ROWB[i] + j
            for ip in range(N):
                lhsT[r, ROWB[ip] + j] = A_g4[j, ip, i]
            lhsT[r, YR + j] = Cv4[j, i]
            bv[r, 0] = Bv4[j, i]
    lhsT_main = lhsT.copy()
    pe_lhsTs = []
    for i in PE_LINES:
        m = np.zeros_like(lhsT)
        rows = slice(ROWB[i], ROWB[i] + IPC)
        m[rows, :] = lhsT[rows, :]
        lhsT_main[rows, :] = 0.0
        pe_lhsTs.append(m)
    return lhsT_main, pe_lhsTs, bv


def _offsets():
    offs = np.zeros((NR, NBLK - 2), np.uint32)
    for r in range(NR):
        offs[r, :] = r * TPAD  # unused rows read their own zero prefix
    for b in range(2, NBLK):
        n0 = L * b
        for i in range(N):
            for j in range(IPC):
                r = ROWB[i] + j
                offs[r, b - 2] = r * TPAD + (PAD + n0 - DELAYS[i])
    return offs


def _build():
    global _BUILT
    if _BUILT is not None:
        return _BUILT
    import concourse.bacc as bacc
    import concourse.bass as bass
    import concourse.mybir as mybir
    import concourse.tile as tile

    fp32 = mybir.dt.float32
    f32r = mybir.dt.float32r
    u32 = mybir.dt.uint32
    nc = bacc.Bacc("TRN2", target_bir_lowering=False, debug=False)
    lhsT_d = nc.dram_tensor("lhsT", [NR, NR], f32r, kind="ExternalInput")
    pe_lhsT_d = [
        nc.dram_tensor(f"pelhsT{k}", [NR, NR], f32r, kind="ExternalInput")
        for k in range(len(PE_LINES))
    ]
    bv_d = nc.dram_tensor("bv", [NR, 1], f32r, kind="ExternalInput")
    offs_d = nc.dram_tensor("offs", [NR, NBLK - 2], u32, kind="ExternalInput")
    patches = _patch_list()
    y_d = nc.dram_tensor("y", [IPC, IR_LEN], f32r, kind="ExternalOutput")
    h_d = nc.dram_tensor("hist", [NR, TPAD], f32r)

    RSLOTS = 9
    RCOLS = RSLOTS * L          # 4500; cols [4500,5000) mirror slot 0

    with tile.TileContext(nc) as tc:
        with tc.tile_pool(name="const", bufs=8) as cpool, \
             tc.tile_pool(name="init", bufs=1) as ipool, \
             tc.tile_pool(name="sg", bufs=8) as spool, \
             tc.tile_pool(name="rg", bufs=1) as rpool, \
             tc.tile_pool(name="ps", bufs=5, space="PSUM") as ppool, \
             tc.tile_pool(name="yb", bufs=2) as ypool:
            lhsT = cpool.tile([NR, NR], f32r)
            nc.sync.dma_start(lhsT[:, :], lhsT_d[:, :])
            pe_lhsT = []
            for k in range(len(PE_LINES)):
                t = cpool.tile([NR, NR], f32r, name=f"pelhsT{k}")
                nc.sync.dma_start(t[:, :], pe_lhsT_d[k][:, :])
                pe_lhsT.append(t)
            offs = cpool.tile([NR, NBLK - 2], u32)
            nc.sync.dma_start(offs[:, :], offs_d[:, :])

            # zero-init history cols [0, PAD+1000); Bv impulse at col PAD.
            z = ipool.tile([NR, PAD + 2 * L], fp32)
            half = (PAD + 2 * L) // 2
            nc.vector.memset(z[:, 0:half], 0.0)
            nc.gpsimd.memset(z[:, half:], 0.0)
            nc.sync.dma_start(z[:, PAD:PAD + 1].bitcast(f32r), bv_d[:, :])
            nc.scalar.dma_start(
                h_d[:, 0:PAD + 2 * L].bitcast(fp32), z[:, :])

            # Stage ring: block m's values at cols (m%9)*L; time u at col
            # u % 4500; [4500,5000) mirrors slot 0 for contiguous windows.
            ring = rpool.tile([NR, RCOLS + L], f32r)
            nc.vector.memset(ring[:, 0:2750].bitcast(fp32), 0.0)
            nc.gpsimd.memset(ring[:, 2750:].bitcast(fp32), 0.0)
            nc.sync.dma_start(ring[:, 0:1], bv_d[:, :])
            nc.sync.dma_start(ring[:, RCOLS:RCOLS + 1], bv_d[:, :])

            for b in range(2, NBLK):
                n0 = L * b
                Xb = PAD + n0 - (K_DEP - 1) * L
                S = spool.tile([NR, L], f32r)
                nc.gpsimd.indirect_dma_start(
                    out=S[:, :], out_offset=None,
                    in_=h_d[:, 0:Xb],
                    in_offset=bass.IndirectOffsetOnAxis(
                        ap=offs[:, b - 2:b - 1], axis=1),
                )
                # plain-copy patches: stale block-(b-2) suffixes of lines 2-4
                # from the ring slot of block b-2 (32-aligned row bases).
                for (i, rel, sc, dc, ln) in patches:
                    pcol = ((b + rel) % RSLOTS) * L
                    rows = slice(ROWB[i], ROWB[i] + IPC)
                    if PATCH_ENG[i] == "scalar":
                        nc.scalar.copy(S[rows, dc:dc + ln],
                                       ring[rows, pcol + sc:pcol + sc + ln])
                    else:
                        nc.vector.tensor_copy(
                            S[rows, dc:dc + ln],
                            ring[rows, pcol + sc:pcol + sc + ln])
                ps = ppool.tile([NR, L], fp32)
                # PE lines 0,1: full 500-wide windows as contiguous
                # (mirror-extended) ring slices; main matmul closes the group.
                for k, i in enumerate(PE_LINES):
                    w = (n0 - DELAYS[i]) % RCOLS
                    nc.tensor.matmul(ps[:, :], pe_lhsT[k][:, :],
                                     ring[:, w:w + L],
                                     start=(k == 0), stop=False)
                nc.tensor.matmul(ps[:, :], lhsT[:, :], S[:, :],
                                 start=(len(PE_LINES) == 0), stop=True)
                rc = (b % RSLOTS) * L
                if b % 2 == 0:
                    nc.vector.tensor_copy(ring[:, rc:rc + L], ps[:, :])
                else:
                    nc.scalar.copy(ring[:, rc:rc + L], ps[:, :])
                if rc == 0:
                    nc.scalar.copy(ring[:, RCOLS:RCOLS + L], ps[:, :])
                nc.sync.dma_start(h_d[:, PAD + n0:PAD + n0 + L],
                                  ring[:, rc:rc + L])

                CH = 12000
                if b >= 25 and (b - 25) % 24 == 0 and (k := (b - 25) // 24) < 3:
                    yb = ypool.tile([IPC, CH], f32r)
                    nc.scalar.dma_start(
                        yb[:, :],
                        h_d[YR:YR + IPC, PAD + k * CH:PAD + (k + 1) * CH])
                    nc.scalar.dma_start(y_d[:, k * CH:(k + 1) * CH], yb[:, :])
                if b >= NBLK - 2:
                    nc.scalar.dma_start(
                        y_d[:, n0:n0 + L], ring[YR:YR + IPC, rc:rc + L])

            yb = ypool.tile([IPC, CH], f32r)
            span = IR_LEN - L * 2 - 3 * CH
            nc.scalar.dma_start(
                yb[:, 0:span],
                h_d[YR:YR + IPC, PAD + 3 * CH:PAD + 3 * CH + span])
            nc.scalar.dma_start(y_d[:, 3 * CH:3 * CH + span], yb[:, 0:span])
    nc.compile()
    _BUILT = nc
    return nc


def _in_maps(A_g, Bv, Cv):
    offs = _offsets()
    maps = []
    for c in range(NCORES):
        sl = slice(c * IPC, (c + 1) * IPC)
        lhsT_main, pe_lhsTs, bv = _core_inputs(A_g[sl], Bv[sl], Cv[sl])
        m = {"lhsT": lhsT_main, "bv": bv, "offs": offs}
        for k, t in enumerate(pe_lhsTs):
            m[f"pelhsT{k}"] = t
        maps.append(m)
    return maps


def kernel(x, WA, bA, WB, bB, WC, bC):
    from concourse import bass_utils

    A_g, Bv, Cv = _prologue(x, WA, bA, WB, bB, WC, bC)
    in_maps = _in_maps(A_g, Bv, Cv)
    nc = _build()
    res = bass_utils.run_bass_kernel_spmd(nc, in_maps, core_ids=list(range(NCORES)))
    y = np.concatenate([res.results[k]["y"] for k in range(NCORES)], axis=0)
    return y[:, None, :].astype(np.float32)
